# revision 2
# baseline (speedup 1.0000x reference)
"""Trainium2 Bass kernel for per-node temporal graph conv (LCN) — v3.

Math (matches the reference): for each node v with neighbor list idx[v]
(chain graph: v-1, v, v+1, masked at the ends),
    out[n,o,v,t] = b[v,o] + sum_{k,c,kt} x_pad[n,c,idx[v,k],t+kt] * Wm[v,o,c,k,kt]

Strategy: data-parallel over batch N across 8 cores (2 samples each);
weights/bias replicated. Host pre-packs x into the exact SBUF layout in
bf16 (node-pair blocks of 514 cols: 512 time steps + 2 zero pads;
partitions 0-63 = odd node 2j-1, 64-127 = even node 2j). Outputs are
computed per node pair (v=2m, 2m+1) stacked on the PSUM partition dim:
6 accumulating bf16 matmuls (3 temporal taps x 2 source blocks) per
pair. Sample loop is OUTER so sample 0's matmuls only wait on the head
of the input stream; inputs are split across both HWDGE rings
(sync: x, scalar: weights) smallest-chunk-first so the PE starts ~4us
in. Bias is fused into the PSUM->SBUF copy (bf16); outputs stream out
in per-sample chunks with a tiny final chunk to shorten the tail.
"""

import numpy as np
import ml_dtypes

import concourse.bacc as bacc
import concourse.mybir as mybir
from concourse.tile import TileContext
from concourse.bass_utils import run_bass_kernel_spmd

V, K, CIN, COUT, N, T, TK = 25, 3, 64, 64, 16, 512, 3
NCORES = 8
NPER = N // NCORES          # samples per core
TP = T + 2                  # block width incl. temporal zero pads
NB = (V + 1) // 2           # node-pair blocks
XW = NB * TP                # x cols per sample in SBUF
YW = NB * T                 # y cols per sample in SBUF

_BF16 = mybir.dt.bfloat16
_F32 = mybir.dt.float32


def _taps(m):
    return [(kt, mmi) for kt in range(TK) for mmi in range(2) if 2 * m + mmi < V]


# pair-major slot table: slot_of[(m, i)] -> column block in wl
_SLOT_BASE = {}
_NSLOT = 0
for _m in range(NB):
    _SLOT_BASE[_m] = _NSLOT
    _NSLOT += len(_taps(_m))

# output chunk boundaries (in pairs); tiny last chunk shortens the tail
_YCHUNKS = [(0, 4), (4, 8), (8, 12), (12, 13)]
_NWARM = 24  # PE warm-up matmuls (keep HAM at 8/8 during the input stream)

_cache = {}


def _build_program():
    nc = bacc.Bacc("TRN2", num_devices=NCORES)
    x_in = nc.dram_tensor("x", [NPER, 128, XW], _BF16, kind="ExternalInput")
    wl_in = nc.dram_tensor("wl", [128, _NSLOT * 128], _BF16, kind="ExternalInput")
    b_in = nc.dram_tensor("bias", [128, NB], _F32, kind="ExternalInput")
    y_out = nc.dram_tensor("y", [NPER, 128, YW], _BF16, kind="ExternalOutput")

    with TileContext(nc) as tc:
        with (
            tc.tile_pool(name="w", bufs=1) as wp,
            tc.tile_pool(name="x", bufs=1) as xp,
            tc.tile_pool(name="ps", bufs=7, space="PSUM") as pp,
            tc.tile_pool(name="warm", bufs=1, space="PSUM") as wmp,
            tc.tile_pool(name="o", bufs=1) as op,
        ):
            b_sb = wp.tile([128, NB], _F32, tag="bias")
            wl_sb = wp.tile([128, _NSLOT * 128], _BF16, tag="wl")
            garb = wp.tile([128, 512], _BF16, tag="garb")  # never written
            xs = [
                xp.tile([128, XW], _BF16, tag=f"x{n}", name=f"x{n}")
                for n in range(NPER)
            ]
            ys = [
                op.tile([128, YW], _BF16, tag=f"y{n}", name=f"y{n}")
                for n in range(NPER)
            ]

            # PE warm-up: matmuls on garbage data into a dead PSUM bank.
            # No DMA dependencies, so they run during the input stream and
            # hold the HAM clock gate at 8/8 until real matmuls are ready.
            wps = wmp.tile([128, 512], _F32, tag="warm")
            nc.vector.memset(garb[:, :], 0.0)
            for i in range(_NWARM):
                nc.tensor.matmul(
                    wps[:, :],
                    lhsT=garb[:, 0:128],
                    rhs=garb[:, :],
                    start=(i == 0),
                    stop=(i == _NWARM - 1),
                )

            # single HWDGE ring (sync) for all inputs, in priority order:
            # first-needed chunks first; x1 last (only needed ~16us in).
            def _wl_dma(c0, c1):
                s0 = _SLOT_BASE[c0] * 128
                s1 = _SLOT_BASE[c1] * 128 if c1 < NB else _NSLOT * 128
                nc.sync.dma_start(out=wl_sb[:, s0:s1], in_=wl_in[:, s0:s1])

            def _x_dma(n, c0, c1):
                nc.sync.dma_start(
                    out=xs[n][:, c0 * TP : c1 * TP],
                    in_=x_in[n, :, c0 * TP : c1 * TP],
                )

            nc.sync.dma_start(out=b_sb[:, :], in_=b_in[:, :])
            _wl_dma(0, 2)
            _x_dma(0, 0, 3)
            _x_dma(0, 3, 8)
            _wl_dma(2, 7)
            _x_dma(0, 8, 13)
            _wl_dma(7, 13)
            _x_dma(1, 0, 7)
            _x_dma(1, 7, 13)

            ychunk = {c1: (c0, c1) for c0, c1 in _YCHUNKS}
            for n in range(NPER):
                for m in range(NB):
                    taps = _taps(m)
                    ps = pp.tile([128, 512], _F32, name=f"ps{n}_{m}", tag="ps")
                    for i, (kt, mmi) in enumerate(taps):
                        slot = _SLOT_BASE[m] + i
                        col = (m + mmi) * TP + kt
                        nc.tensor.matmul(
                            ps[:, :],
                            lhsT=wl_sb[:, slot * 128 : (slot + 1) * 128],
                            rhs=xs[n][:, col : col + 512],
                            start=(i == 0),
                            stop=(i == len(taps) - 1),
                        )
                    nc.vector.tensor_scalar_add(
                        out=ys[n][:, m * 512 : (m + 1) * 512],
                        in0=ps[:, :],
                        scalar1=b_sb[:, m : m + 1],
                    )
                    if m + 1 in ychunk:
                        # outputs ride the scalar HWDGE ring so they never
                        # queue behind the input stream on the sync ring.
                        c0, c1 = ychunk[m + 1]
                        nc.scalar.dma_start(
                            out=y_out[n, :, c0 * 512 : c1 * 512],
                            in_=ys[n][:, c0 * 512 : c1 * 512],
                        )

    nc.compile()
    return nc


def _prep_weights(W, b, idx, mask):
    W = np.asarray(W, np.float32)
    b = np.asarray(b, np.float32)
    idx = np.asarray(idx)
    mask = np.asarray(mask)
    Wm = np.where(mask[:, None, None, :, None], W, 0.0)  # [V,O,C,K,TK]
    W4 = np.zeros((V, V, COUT, CIN, TK), np.float32)
    for v in range(V):
        for k in range(K):
            if mask[v, k]:
                W4[v, idx[v, k]] = Wm[v, :, :, k, :]
    wl = np.zeros((128, _NSLOT * 128), np.float32)
    for m in range(NB):
        for i, (kt, mmi) in enumerate(_taps(m)):
            slot = _SLOT_BASE[m] + i
            blk = m + mmi
            for uh, u in ((0, 2 * blk - 1), (1, 2 * blk)):
                for vloc in range(2):
                    v = 2 * m + vloc
                    if 0 <= u < V and v < V:
                        # lhsT[64*uh + c, 64*vloc + o] = W4[v,u,o,c,kt]
                        wl[
                            64 * uh : 64 * uh + 64,
                            slot * 128 + 64 * vloc : slot * 128 + 64 * vloc + 64,
                        ] = W4[v, u, :, :, kt].T
    bias = np.zeros((128, NB), np.float32)
    for m in range(NB):
        for vloc in range(2):
            if 2 * m + vloc < V:
                bias[64 * vloc : 64 * vloc + 64, m] = b[2 * m + vloc]
    return wl.astype(ml_dtypes.bfloat16), bias


def _prep_x(x):
    """[N, C, V, T] f32 -> [N, 128, NB*TP] bf16 in the SBUF block layout."""
    x = np.asarray(x, np.float32)
    xs = np.zeros((N, 128, NB, TP), np.float32)
    xs[:, 64:128, :, 1 : T + 1] = x[:, :, ::2, :]
    xs[:, 0:64, 1:NB, 1 : T + 1] = x[:, :, 1::2, :]
    return np.ascontiguousarray(xs.reshape(N, 128, NB * TP)).astype(ml_dtypes.bfloat16)


def _unpack_y(yc):
    """list of [NPER, 128, NB*T] bf16 -> [N, O, V, T] f32."""
    y = np.stack([np.asarray(c) for c in yc]).astype(np.float32)  # [8,NPER,128,YW]
    y = y.reshape(N, 2, 64, NB, T)          # [n, vloc, o, m, t]
    y = y.transpose(0, 2, 3, 1, 4)          # [n, o, m, vloc, t]
    return np.ascontiguousarray(y.reshape(N, 64, 2 * NB, T)[:, :, :V, :])


def kernel(x, W, b, idx, mask):
    if "nc" not in _cache:
        _cache["nc"] = _build_program()
    nc = _cache["nc"]
    wl, bias = _prep_weights(W, b, idx, mask)
    xp = _prep_x(x)
    in_maps = [
        {"x": np.ascontiguousarray(xp[c * NPER : (c + 1) * NPER]), "wl": wl, "bias": bias}
        for c in range(NCORES)
    ]
    res = run_bass_kernel_spmd(nc, in_maps, list(range(NCORES)))
    return _unpack_y([res.results[c]["y"] for c in range(NCORES)])
